# revision 7
# baseline (speedup 1.0000x reference)
"""CorrBlock1d sampling kernel for Trainium2 (Bass/Tile), 8-core data-parallel.

Strategy
--------
Per row n with coord c: level-l output is a 9-tap lerp over the 10-element
window corr_l[n, ib_l-4 : ib_l+6], ib_l = floor(c/2^l), shared fractional
weight f_l = frac(c/2^l).  All taps outside [0, Wl-1] read as zero.

The only fine-grained gather primitive on TRN2 (gpsimd indirect DMA) costs
~1.1us per call and serves at most 128 descriptors (one per SBUF partition,
each a contiguous src block).  So the design minimizes *descriptors per row*:

Host interleaves the pyramid into two arrays per row (data-independent):
  P01 slot w  (w in [-9, 266]):  (corr0[w],  corr1[w>>1])    552 f32/row
  P23 slot w2 (w2 in [-9, 74]):  (corr2[w2], corr3[w2>>1])   168 f32/row
with zeros outside valid index ranges (this also implements the reference's
zero padding, so no on-chip masking is needed).

One 40-f32 descriptor anchored at slot ib0-9 of P01 then contains BOTH the
level-0 and level-1 windows at *static* positions: corr0 taps at slot j+5
comp 0; corr1 taps at slot 2j+1 comp 1 (reading the w>>1 component at
odd-aligned stride-2 positions absorbs the anchor's low bit exactly:
(ib0-8+2j)>>1 = ib1-4+j for any parity of ib0).  Same for P23 anchored at
ib2-9 (levels 2,3).  Hence 2 descriptors/row -> 256 indirect calls/core.

Row m = t*128+p lives on partition p, tile-column t; host pre-transposes
coords and un-transposes the [128, NT*36] output.
"""

import numpy as np

import concourse.bacc as bacc
import concourse.bass as bass
import concourse.mybir as mybir
import concourse.tile as tile
from concourse.bass_utils import run_bass_kernel_spmd

F32 = mybir.dt.float32
I32 = mybir.dt.int32
OP = mybir.AluOpType
AP = bass.AP

P = 128
NCORES = 8
B, H, W = 8, 64, 256
N = B * H * W              # 131072 rows
R = N // NCORES            # 16384 rows per core
NT = R // P                # 128 tiles of 128 rows
K = 9
CH = 36
D = 40                     # f32 fetched per descriptor (20 slots x 2)
PAD = 9                    # slots of front padding in P01/P23
S01 = 276                  # slots per row in P01  (w in [-9, 266])
S23 = 84                   # slots per row in P23  (w2 in [-9, 74])
MAGIC = float(1 << 23)


def _floor(nc, pool, x, chunk, tag):
    """xb = floor(x) for x >= 0 via rne(+2^23) then fix-up."""
    t = pool.tile([P, chunk], F32, tag=f"t{tag}")
    nc.vector.tensor_scalar_add(t[:], x[:], MAGIC)
    y = pool.tile([P, chunk], F32, tag=f"y{tag}")
    nc.vector.tensor_scalar_sub(y[:], t[:], MAGIC)
    gt = pool.tile([P, chunk], F32, tag=f"gt{tag}")
    nc.vector.tensor_tensor(gt[:], y[:], x[:], OP.is_gt)
    xb = pool.tile([P, chunk], F32, tag=f"xb{tag}")
    nc.vector.tensor_sub(xb[:], y[:], gt[:])
    return xb


def _slice(win, chunk, span, start, step, count):
    """3D AP [P, chunk, count]: per tile-block of `span` f32, elements
    start, start+step, ... (manual AP construction)."""
    w = win[:]
    return AP(w.tensor, w.offset + start,
              [list(w.ap[0]), [span, chunk], [step, count]])


def build_nc(r=R, chunk=32):
    nt = r // P
    chunk = min(chunk, nt)
    ngrp = nt // chunk

    nc = bacc.Bacc("TRN2", target_bir_lowering=False, debug=False)

    coords = nc.dram_tensor("coords", [P, nt], F32, kind="ExternalInput")
    mrow = nc.dram_tensor("mrow", [P, nt], F32, kind="ExternalInput")
    p01 = nc.dram_tensor("p01", [r * 2 * S01], F32, kind="ExternalInput")
    p23 = nc.dram_tensor("p23", [r * 2 * S23], F32, kind="ExternalInput")
    out = nc.dram_tensor("out", [P, nt * CH], F32, kind="ExternalOutput")
    p01v = p01[:].rearrange("(a b) -> a b", b=1)
    p23v = p23[:].rearrange("(a b) -> a b", b=1)

    with tile.TileContext(nc) as tc:
        with (
            tc.tile_pool(name="const", bufs=1) as cpool,
            tc.tile_pool(name="idx", bufs=1) as ipool,
            tc.tile_pool(name="wide", bufs=2) as wpool,
            tc.tile_pool(name="outp", bufs=2) as opool,
        ):
            coords_t = cpool.tile([P, nt], F32, tag="coords")
            nc.sync.dma_start(out=coords_t[:], in_=coords[:])
            mrow_t = cpool.tile([P, nt], F32, tag="mrow")
            nc.sync.dma_start(out=mrow_t[:], in_=mrow[:])

            # --- whole-core index math (once) ---
            ibs, fracs, w0s = [], [], []
            for l in range(4):
                x = ipool.tile([P, nt], F32, tag=f"x{l}")
                nc.vector.tensor_scalar_mul(x[:], coords_t[:], 1.0 / (1 << l))
                ib = _floor(nc, ipool, x, nt, f"f{l}")
                f = ipool.tile([P, nt], F32, tag=f"fr{l}")
                nc.vector.tensor_sub(f[:], x[:], ib[:])
                w0 = ipool.tile([P, nt], F32, tag=f"w0{l}")
                nc.vector.tensor_scalar(w0[:], f[:], -1.0, 1.0, OP.mult, OP.add)
                ibs.append(ib)
                fracs.append(f)
                w0s.append(w0)

            idxs = []
            for (nm, ib, span) in (("01", ibs[0], 2 * S01),
                                   ("23", ibs[2], 2 * S23)):
                ib2x = ipool.tile([P, nt], F32, tag=f"ib2x{nm}")
                nc.vector.tensor_add(ib2x[:], ib[:], ib[:])
                idf = ipool.tile([P, nt], F32, tag=f"idf{nm}")
                nc.vector.scalar_tensor_tensor(
                    idf[:], in0=mrow_t[:], scalar=float(span),
                    in1=ib2x[:], op0=OP.mult, op1=OP.add)
                idi = ipool.tile([P, nt], I32, tag=f"idi{nm}")
                nc.vector.tensor_copy(idi[:], idf[:])
                idxs.append(idi)
            idx01, idx23 = idxs

            for g in range(ngrp):
                g0 = g * chunk
                out_t = opool.tile([P, chunk * CH], F32, tag="out")
                wins = []
                for (nm, idi, srcv) in (("01", idx01, p01v),
                                        ("23", idx23, p23v)):
                    win = wpool.tile([P, chunk * D], F32, tag=f"win{nm}")
                    for t in range(chunk):
                        nc.gpsimd.indirect_dma_start(
                            out=win[:, t * D:(t + 1) * D],
                            out_offset=None,
                            in_=srcv,
                            in_offset=bass.IndirectOffsetOnAxis(
                                ap=idi[:, g0 + t:g0 + t + 1], axis=0),
                        )
                    wins.append(win)

                # taps: level 0/2 -> slot j+5 comp 0 => flat 2j+10 (step 2)
                #       level 1/3 -> slot 2j+1 comp 1 => flat 4j+3 (step 4)
                for l in range(4):
                    win = wins[0] if l < 2 else wins[1]
                    fine = (l % 2 == 0)
                    start, step = (10, 2) if fine else (3, 4)
                    sL = _slice(win, chunk, D, start, step, K)
                    sR = _slice(win, chunk, D, start + step, step, K)
                    fb = fracs[l][:, g0:g0 + chunk] \
                        .rearrange("p (t o) -> p t o", o=1) \
                        .to_broadcast([P, chunk, K])
                    wb = w0s[l][:, g0:g0 + chunk] \
                        .rearrange("p (t o) -> p t o", o=1) \
                        .to_broadcast([P, chunk, K])
                    t0 = wpool.tile([P, chunk * K], F32, tag=f"t0{l}")
                    t03 = t0[:].rearrange("p (t w) -> p t w", w=K)
                    nc.vector.tensor_tensor(t03, sL, wb, OP.mult)
                    t1 = wpool.tile([P, chunk * K], F32, tag=f"t1{l}")
                    t13 = t1[:].rearrange("p (t w) -> p t w", w=K)
                    nc.vector.tensor_tensor(t13, sR, fb, OP.mult)
                    o3 = out_t[:].rearrange("p (t c) -> p t c", c=CH)
                    nc.vector.tensor_tensor(
                        o3[:, :, l * K:(l + 1) * K], t03, t13, OP.add)

                nc.sync.dma_start(
                    out=out[:, g0 * CH:(g0 + chunk) * CH], in_=out_t[:])

    nc.compile()
    return nc


def _interleave(fine, coarse, nslots):
    """[r, Wf], [r, Wc] -> [r, nslots*2]: slot w (w = idx-PAD) holds
    (fine[w], coarse[w>>1]), zeros outside valid ranges."""
    r, wf = fine.shape
    wc = coarse.shape[1]
    w = np.arange(nslots) - PAD
    out = np.zeros((r, nslots, 2), np.float32)
    m0 = (w >= 0) & (w < wf)
    out[:, m0, 0] = fine[:, w[m0]]
    wh = np.floor_divide(w, 2)
    m1 = (wh >= 0) & (wh < wc)
    out[:, m1, 1] = coarse[:, wh[m1]]
    return out.reshape(r, nslots * 2)


def make_in_maps(centroids_coords, corr_list, r=R):
    nt = r // P
    c = np.ascontiguousarray(centroids_coords[:, 0], dtype=np.float32).reshape(-1)
    mrow = np.arange(r, dtype=np.float32).reshape(nt, P).T.copy()
    ncores = c.size // r
    in_maps = []
    for k in range(ncores):
        sl = slice(k * r, (k + 1) * r)
        in_maps.append({
            "coords": c[sl].reshape(nt, P).T.copy(),
            "mrow": mrow,
            "p01": _interleave(np.asarray(corr_list[0][sl], np.float32),
                               np.asarray(corr_list[1][sl], np.float32),
                               S01).ravel(),
            "p23": _interleave(np.asarray(corr_list[2][sl], np.float32),
                               np.asarray(corr_list[3][sl], np.float32),
                               S23).ravel(),
        })
    return in_maps


_NC_CACHE = {}
LAST_RESULTS = None


def kernel(centroids_coords, corr0, corr1, corr2, corr3,
           trace=False, tmpdir=None):
    global LAST_RESULTS
    centroids_coords = np.asarray(centroids_coords, dtype=np.float32)
    corrs = [np.asarray(x, dtype=np.float32) for x in (corr0, corr1, corr2, corr3)]

    if "nc" not in _NC_CACHE:
        _NC_CACHE["nc"] = build_nc()
    nc = _NC_CACHE["nc"]

    in_maps = make_in_maps(centroids_coords, corrs)
    res = run_bass_kernel_spmd(nc, in_maps, list(range(NCORES)),
                               trace=trace, tmpdir=tmpdir)
    LAST_RESULTS = res

    parts = []
    for k in range(NCORES):
        o = res.results[k]["out"]
        parts.append(o.reshape(P, NT, CH).transpose(1, 0, 2).reshape(R, CH))
    full = np.concatenate(parts, axis=0)
    return np.ascontiguousarray(
        full.reshape(B, H, W, CH).transpose(0, 3, 1, 2))


# revision 9
# speedup vs baseline: 1.0076x; 1.0076x over previous
"""CorrBlock1d sampling kernel for Trainium2 (Bass/Tile), 8-core data-parallel.

Strategy
--------
Per row n with coord c: level-l output is a 9-tap lerp over the 10-element
window corr_l[n, ib_l-4 : ib_l+6], ib_l = floor(c/2^l), shared fractional
weight f_l = frac(c/2^l).  All taps outside [0, Wl-1] read as zero.

The only fine-grained gather primitive on TRN2 (gpsimd indirect DMA) costs
~1.1us per call and serves at most 128 descriptors (one per SBUF partition,
each a contiguous src block).  So the design minimizes *descriptors per row*:

Host interleaves the pyramid into two arrays per row (data-independent):
  P01 slot w  (w in [-9, 266]):  (corr0[w],  corr1[w>>1])    552 f32/row
  P23 slot w2 (w2 in [-9, 74]):  (corr2[w2], corr3[w2>>1])   168 f32/row
with zeros outside valid index ranges (this also implements the reference's
zero padding, so no on-chip masking is needed).

One 40-f32 descriptor anchored at slot ib0-9 of P01 then contains BOTH the
level-0 and level-1 windows at *static* positions: corr0 taps at slot j+5
comp 0; corr1 taps at slot 2j+1 comp 1 (reading the w>>1 component at
odd-aligned stride-2 positions absorbs the anchor's low bit exactly:
(ib0-8+2j)>>1 = ib1-4+j for any parity of ib0).  Same for P23 anchored at
ib2-9 (levels 2,3).  Hence 2 descriptors/row -> 256 indirect calls/core.

Row m = t*128+p lives on partition p, tile-column t; host pre-transposes
coords and un-transposes the [128, NT*36] output.
"""

import numpy as np

import concourse.bacc as bacc
import concourse.bass as bass
import concourse.mybir as mybir
import concourse.tile as tile
from concourse.bass_utils import run_bass_kernel_spmd

F32 = mybir.dt.float32
I32 = mybir.dt.int32
OP = mybir.AluOpType
AP = bass.AP

P = 128
NCORES = 8
B, H, W = 8, 64, 256
N = B * H * W              # 131072 rows
R = N // NCORES            # 16384 rows per core
NT = R // P                # 128 tiles of 128 rows
K = 9
CH = 36
D = 40                     # f32 fetched per descriptor (20 slots x 2)
PAD = 9                    # slots of front padding in P01/P23
S01 = 276                  # slots per row in P01  (w in [-9, 266])
S23 = 84                   # slots per row in P23  (w2 in [-9, 74])
MAGIC = float(1 << 23)


def _floor(nc, pool, x, chunk, tag):
    """xb = floor(x) for x >= 0 via rne(+2^23) then fix-up."""
    t = pool.tile([P, chunk], F32, tag=f"t{tag}")
    nc.vector.tensor_scalar_add(t[:], x[:], MAGIC)
    y = pool.tile([P, chunk], F32, tag=f"y{tag}")
    nc.vector.tensor_scalar_sub(y[:], t[:], MAGIC)
    gt = pool.tile([P, chunk], F32, tag=f"gt{tag}")
    nc.vector.tensor_tensor(gt[:], y[:], x[:], OP.is_gt)
    xb = pool.tile([P, chunk], F32, tag=f"xb{tag}")
    nc.vector.tensor_sub(xb[:], y[:], gt[:])
    return xb


def _slice(win, chunk, span, start, step, count):
    """3D AP [P, chunk, count]: per tile-block of `span` f32, elements
    start, start+step, ... (manual AP construction)."""
    w = win[:]
    return AP(w.tensor, w.offset + start,
              [list(w.ap[0]), [span, chunk], [step, count]])


def build_nc(r=R, chunk=32):
    nt = r // P
    chunk = min(chunk, nt)
    ngrp = nt // chunk

    nc = bacc.Bacc("TRN2", target_bir_lowering=False, debug=False,
                   num_swdge_queues=4)

    coords = nc.dram_tensor("coords", [P, nt], F32, kind="ExternalInput")
    mrow = nc.dram_tensor("mrow", [P, nt], F32, kind="ExternalInput")
    p01 = nc.dram_tensor("p01", [r * 2 * S01], F32, kind="ExternalInput")
    p23 = nc.dram_tensor("p23", [r * 2 * S23], F32, kind="ExternalInput")
    out = nc.dram_tensor("out", [P, nt * CH], F32, kind="ExternalOutput")
    p01v = p01[:].rearrange("(a b) -> a b", b=1)
    p23v = p23[:].rearrange("(a b) -> a b", b=1)

    with tile.TileContext(nc) as tc:
        with (
            tc.tile_pool(name="const", bufs=1) as cpool,
            tc.tile_pool(name="idx", bufs=1) as ipool,
            tc.tile_pool(name="wide", bufs=2) as wpool,
            tc.tile_pool(name="outp", bufs=2) as opool,
        ):
            coords_t = cpool.tile([P, nt], F32, tag="coords")
            nc.sync.dma_start(out=coords_t[:], in_=coords[:])
            mrow_t = cpool.tile([P, nt], F32, tag="mrow")
            nc.sync.dma_start(out=mrow_t[:], in_=mrow[:])

            # --- whole-core index math (once) ---
            ibs, fracs, w0s = [], [], []
            for l in range(4):
                x = ipool.tile([P, nt], F32, tag=f"x{l}")
                nc.vector.tensor_scalar_mul(x[:], coords_t[:], 1.0 / (1 << l))
                ib = _floor(nc, ipool, x, nt, f"f{l}")
                f = ipool.tile([P, nt], F32, tag=f"fr{l}")
                nc.vector.tensor_sub(f[:], x[:], ib[:])
                w0 = ipool.tile([P, nt], F32, tag=f"w0{l}")
                nc.vector.tensor_scalar(w0[:], f[:], -1.0, 1.0, OP.mult, OP.add)
                ibs.append(ib)
                fracs.append(f)
                w0s.append(w0)

            idxs = []
            for (nm, ib, span) in (("01", ibs[0], 2 * S01),
                                   ("23", ibs[2], 2 * S23)):
                ib2x = ipool.tile([P, nt], F32, tag=f"ib2x{nm}")
                nc.vector.tensor_add(ib2x[:], ib[:], ib[:])
                idf = ipool.tile([P, nt], F32, tag=f"idf{nm}")
                nc.vector.scalar_tensor_tensor(
                    idf[:], in0=mrow_t[:], scalar=float(span),
                    in1=ib2x[:], op0=OP.mult, op1=OP.add)
                idi = ipool.tile([P, nt], I32, tag=f"idi{nm}")
                nc.vector.tensor_copy(idi[:], idf[:])
                idxs.append(idi)
            idx01, idx23 = idxs

            for g in range(ngrp):
                g0 = g * chunk
                out_t = opool.tile([P, chunk * CH], F32, tag="out")
                wins = []
                for (nm, idi, srcv) in (("01", idx01, p01v),
                                        ("23", idx23, p23v)):
                    win = wpool.tile([P, chunk * D], F32, tag=f"win{nm}")
                    for t in range(chunk):
                        inst = nc.gpsimd.indirect_dma_start(
                            out=win[:, t * D:(t + 1) * D],
                            out_offset=None,
                            in_=srcv,
                            in_offset=bass.IndirectOffsetOnAxis(
                                ap=idi[:, g0 + t:g0 + t + 1], axis=0),
                        )
                        q = t % 4
                        if q:
                            inst.ins.queue = f"qPoolDynamic{q}"
                    wins.append(win)

                # taps: level 0/2 -> slot j+5 comp 0 => flat 2j+10 (step 2)
                #       level 1/3 -> slot 2j+1 comp 1 => flat 4j+3 (step 4)
                for l in range(4):
                    win = wins[0] if l < 2 else wins[1]
                    fine = (l % 2 == 0)
                    start, step = (10, 2) if fine else (3, 4)
                    sL = _slice(win, chunk, D, start, step, K)
                    sR = _slice(win, chunk, D, start + step, step, K)
                    fb = fracs[l][:, g0:g0 + chunk] \
                        .rearrange("p (t o) -> p t o", o=1) \
                        .to_broadcast([P, chunk, K])
                    wb = w0s[l][:, g0:g0 + chunk] \
                        .rearrange("p (t o) -> p t o", o=1) \
                        .to_broadcast([P, chunk, K])
                    t0 = wpool.tile([P, chunk * K], F32, tag=f"t0{l}")
                    t03 = t0[:].rearrange("p (t w) -> p t w", w=K)
                    nc.vector.tensor_tensor(t03, sL, wb, OP.mult)
                    t1 = wpool.tile([P, chunk * K], F32, tag=f"t1{l}")
                    t13 = t1[:].rearrange("p (t w) -> p t w", w=K)
                    nc.vector.tensor_tensor(t13, sR, fb, OP.mult)
                    o3 = out_t[:].rearrange("p (t c) -> p t c", c=CH)
                    nc.vector.tensor_tensor(
                        o3[:, :, l * K:(l + 1) * K], t03, t13, OP.add)

                nc.sync.dma_start(
                    out=out[:, g0 * CH:(g0 + chunk) * CH], in_=out_t[:])

    nc.compile()
    return nc


def _interleave(fine, coarse, nslots):
    """[r, Wf], [r, Wc] -> [r, nslots*2]: slot w (w = idx-PAD) holds
    (fine[w], coarse[w>>1]), zeros outside valid ranges."""
    r, wf = fine.shape
    wc = coarse.shape[1]
    w = np.arange(nslots) - PAD
    out = np.zeros((r, nslots, 2), np.float32)
    m0 = (w >= 0) & (w < wf)
    out[:, m0, 0] = fine[:, w[m0]]
    wh = np.floor_divide(w, 2)
    m1 = (wh >= 0) & (wh < wc)
    out[:, m1, 1] = coarse[:, wh[m1]]
    return out.reshape(r, nslots * 2)


def make_in_maps(centroids_coords, corr_list, r=R):
    nt = r // P
    c = np.ascontiguousarray(centroids_coords[:, 0], dtype=np.float32).reshape(-1)
    mrow = np.arange(r, dtype=np.float32).reshape(nt, P).T.copy()
    ncores = c.size // r
    in_maps = []
    for k in range(ncores):
        sl = slice(k * r, (k + 1) * r)
        in_maps.append({
            "coords": c[sl].reshape(nt, P).T.copy(),
            "mrow": mrow,
            "p01": _interleave(np.asarray(corr_list[0][sl], np.float32),
                               np.asarray(corr_list[1][sl], np.float32),
                               S01).ravel(),
            "p23": _interleave(np.asarray(corr_list[2][sl], np.float32),
                               np.asarray(corr_list[3][sl], np.float32),
                               S23).ravel(),
        })
    return in_maps


_NC_CACHE = {}
LAST_RESULTS = None


def kernel(centroids_coords, corr0, corr1, corr2, corr3,
           trace=False, tmpdir=None):
    global LAST_RESULTS
    centroids_coords = np.asarray(centroids_coords, dtype=np.float32)
    corrs = [np.asarray(x, dtype=np.float32) for x in (corr0, corr1, corr2, corr3)]

    if "nc" not in _NC_CACHE:
        _NC_CACHE["nc"] = build_nc()
    nc = _NC_CACHE["nc"]

    in_maps = make_in_maps(centroids_coords, corrs)
    res = run_bass_kernel_spmd(nc, in_maps, list(range(NCORES)),
                               trace=trace, tmpdir=tmpdir)
    LAST_RESULTS = res

    parts = []
    for k in range(NCORES):
        o = res.results[k]["out"]
        parts.append(o.reshape(P, NT, CH).transpose(1, 0, 2).reshape(R, CH))
    full = np.concatenate(parts, axis=0)
    return np.ascontiguousarray(
        full.reshape(B, H, W, CH).transpose(0, 3, 1, 2))


# revision 10
# speedup vs baseline: 1.8206x; 1.8069x over previous
"""Q6 layout: ONE descriptor per row fetches all 4 levels' windows.

Slot w1 (anchor a = ib1-9, fetch 20 slots x 6 f32 = 120 f32):
  q0=corr0[2w1] q1=corr0[2w1+1] q2=corr1[w1] q3=corr2[w1>>1]
  q4=corr3[(w1>>2)-2] q5=corr3[(w1>>2)+3]
Static taps (flat = 6*pos+comp): l1: 6j+32; l2: 12j+9; l3: 24j+10 (j<5),
24(j-5)+11 (j>=5).  l0 via E0[i]=flat 6i+42, E1[i]=6i+43 and parity blend:
  outEven[i] = E0[i]*a + E1[i]*b + E0[i+1]*g   (channels 0,2,4,6,8)
  outOdd[i]  = E1[i]*a + E0[i+1]*b + E1[i+1]*g (channels 1,3,5,7)
  a = w0*(1-r0), b = f*(1-r0)+w0*r0, g = f*r0,  r0 = ib0-2*ib1.
"""
import numpy as np

import concourse.bacc as bacc
import concourse.bass as bass
import concourse.mybir as mybir
import concourse.tile as tile
from concourse.bass_utils import run_bass_kernel_spmd

F32 = mybir.dt.float32
I32 = mybir.dt.int32
OP = mybir.AluOpType
AP = bass.AP

P = 128
NCORES = 8
B, H, W = 8, 64, 256
N = B * H * W
R = N // NCORES
NT = R // P
K = 9
CH = 36
D = 120
PAD = 9
SQ = 147
MAGIC = float(1 << 23)


def _floor(nc, pool, x, chunk, tag):
    t = pool.tile([P, chunk], F32, tag=f"t{tag}")
    nc.vector.tensor_scalar_add(t[:], x[:], MAGIC)
    y = pool.tile([P, chunk], F32, tag=f"y{tag}")
    nc.vector.tensor_scalar_sub(y[:], t[:], MAGIC)
    gt = pool.tile([P, chunk], F32, tag=f"gt{tag}")
    nc.vector.tensor_tensor(gt[:], y[:], x[:], OP.is_gt)
    xb = pool.tile([P, chunk], F32, tag=f"xb{tag}")
    nc.vector.tensor_sub(xb[:], y[:], gt[:])
    return xb


def _sl(win, chunk, start, step, count):
    w = win[:]
    return AP(w.tensor, w.offset + start,
              [list(w.ap[0]), [D, chunk], [step, count]])


def _osl(out_t, chunk, start, step, count):
    w = out_t[:]
    return AP(w.tensor, w.offset + start,
              [list(w.ap[0]), [CH, chunk], [step, count]])


def build_nc(r=R, chunk=32):
    nt = r // P
    chunk = min(chunk, nt)
    ngrp = nt // chunk

    nc = bacc.Bacc("TRN2", target_bir_lowering=False, debug=False,
                   num_swdge_queues=4)
    coords = nc.dram_tensor("coords", [P, nt], F32, kind="ExternalInput")
    mrow = nc.dram_tensor("mrow", [P, nt], F32, kind="ExternalInput")
    q6 = nc.dram_tensor("q6", [r * 6 * SQ], F32, kind="ExternalInput")
    out = nc.dram_tensor("out", [P, nt * CH], F32, kind="ExternalOutput")
    q6v = q6[:].rearrange("(a b) -> a b", b=1)

    with tile.TileContext(nc) as tc:
        with (
            tc.tile_pool(name="const", bufs=1) as cpool,
            tc.tile_pool(name="idx", bufs=1) as ipool,
            tc.tile_pool(name="wide", bufs=3) as wpool,
            tc.tile_pool(name="outp", bufs=2) as opool,
        ):
            coords_t = cpool.tile([P, nt], F32, tag="coords")
            nc.sync.dma_start(out=coords_t[:], in_=coords[:])
            mrow_t = cpool.tile([P, nt], F32, tag="mrow")
            nc.sync.dma_start(out=mrow_t[:], in_=mrow[:])

            ibs, fracs, w0s = [], [], []
            for l in range(4):
                x = ipool.tile([P, nt], F32, tag=f"x{l}")
                nc.vector.tensor_scalar_mul(x[:], coords_t[:], 1.0 / (1 << l))
                ib = _floor(nc, ipool, x, nt, f"f{l}")
                f = ipool.tile([P, nt], F32, tag=f"fr{l}")
                nc.vector.tensor_sub(f[:], x[:], ib[:])
                w0 = ipool.tile([P, nt], F32, tag=f"w0{l}")
                nc.vector.tensor_scalar(w0[:], f[:], -1.0, 1.0, OP.mult, OP.add)
                ibs.append(ib)
                fracs.append(f)
                w0s.append(w0)

            # gather index: 882*m + 6*ib1
            ib1x6 = ipool.tile([P, nt], F32, tag="ib1x6")
            nc.vector.tensor_scalar_mul(ib1x6[:], ibs[1][:], 6.0)
            idf = ipool.tile([P, nt], F32, tag="idf")
            nc.vector.scalar_tensor_tensor(
                idf[:], in0=mrow_t[:], scalar=float(6 * SQ),
                in1=ib1x6[:], op0=OP.mult, op1=OP.add)
            idi = ipool.tile([P, nt], I32, tag="idi")
            nc.vector.tensor_copy(idi[:], idf[:])

            # l0 parity blend weights
            ib1x2 = ipool.tile([P, nt], F32, tag="ib1x2")
            nc.vector.tensor_add(ib1x2[:], ibs[1][:], ibs[1][:])
            r0 = ipool.tile([P, nt], F32, tag="r0")
            nc.vector.tensor_sub(r0[:], ibs[0][:], ib1x2[:])
            r0m = ipool.tile([P, nt], F32, tag="r0m")
            nc.vector.tensor_scalar(r0m[:], r0[:], -1.0, 1.0, OP.mult, OP.add)
            al = ipool.tile([P, nt], F32, tag="al")
            nc.vector.tensor_mul(al[:], w0s[0][:], r0m[:])
            b1 = ipool.tile([P, nt], F32, tag="b1")
            nc.vector.tensor_mul(b1[:], fracs[0][:], r0m[:])
            b2 = ipool.tile([P, nt], F32, tag="b2")
            nc.vector.tensor_mul(b2[:], w0s[0][:], r0[:])
            be = ipool.tile([P, nt], F32, tag="be")
            nc.vector.tensor_add(be[:], b1[:], b2[:])
            ga = ipool.tile([P, nt], F32, tag="ga")
            nc.vector.tensor_mul(ga[:], fracs[0][:], r0[:])

            def bc(tile_, g0, cnt):
                return tile_[:, g0:g0 + chunk] \
                    .rearrange("p (t o) -> p t o", o=1) \
                    .to_broadcast([P, chunk, cnt])

            for g in range(ngrp):
                g0 = g * chunk
                out_t = opool.tile([P, chunk * CH], F32, tag="out")
                win = wpool.tile([P, chunk * D], F32, tag="win")
                for t in range(chunk):
                    inst = nc.gpsimd.indirect_dma_start(
                        out=win[:, t * D:(t + 1) * D], out_offset=None,
                        in_=q6v,
                        in_offset=bass.IndirectOffsetOnAxis(
                            ap=idi[:, g0 + t:g0 + t + 1], axis=0))
                    q = t % 4
                    if q:
                        inst.ins.queue = f"qPoolDynamic{q}"

                # levels 1..3 standard lerp from static strided taps
                for l, (start, step) in ((1, (32, 6)), (2, (9, 12))):
                    sL = _sl(win, chunk, start, step, K)
                    sR = _sl(win, chunk, start + step, step, K)
                    t0 = wpool.tile([P, chunk * K], F32, tag=f"t0{l}")
                    t03 = t0[:].rearrange("p (t w) -> p t w", w=K)
                    nc.vector.tensor_tensor(t03, sL, bc(w0s[l], g0, K), OP.mult)
                    t1 = wpool.tile([P, chunk * K], F32, tag=f"t1{l}")
                    t13 = t1[:].rearrange("p (t w) -> p t w", w=K)
                    nc.vector.tensor_tensor(t13, sR, bc(fracs[l], g0, K), OP.mult)
                    o3 = out_t[:].rearrange("p (t c) -> p t c", c=CH)
                    nc.vector.tensor_tensor(
                        o3[:, :, l * K:(l + 1) * K], t03, t13, OP.add)

                # level 3: materialize win3 then lerp
                w3t = wpool.tile([P, chunk * 10], F32, tag="w3t")
                w33 = w3t[:].rearrange("p (t w) -> p t w", w=10)
                nc.vector.tensor_copy(w33[:, :, 0:5], _sl(win, chunk, 10, 24, 5))
                nc.vector.tensor_copy(w33[:, :, 5:10], _sl(win, chunk, 11, 24, 5))
                t0 = wpool.tile([P, chunk * K], F32, tag="t03l")
                t03 = t0[:].rearrange("p (t w) -> p t w", w=K)
                nc.vector.tensor_tensor(t03, w33[:, :, 0:9], bc(w0s[3], g0, K),
                                        OP.mult)
                t1 = wpool.tile([P, chunk * K], F32, tag="t13l")
                t13 = t1[:].rearrange("p (t w) -> p t w", w=K)
                nc.vector.tensor_tensor(t13, w33[:, :, 1:10], bc(fracs[3], g0, K),
                                        OP.add if False else OP.mult)
                o3 = out_t[:].rearrange("p (t c) -> p t c", c=CH)
                nc.vector.tensor_tensor(
                    o3[:, :, 27:36], t03, t13, OP.add)

                # level 0: parity blend
                E0a = _sl(win, chunk, 42, 6, 5)      # E0[0..4]
                E0b = _sl(win, chunk, 48, 6, 5)      # E0[1..5]
                E1a = _sl(win, chunk, 43, 6, 5)      # E1[0..4]
                E1b = _sl(win, chunk, 49, 6, 5)      # E1[1..5]
                te = wpool.tile([P, chunk * 5], F32, tag="te")
                te3 = te[:].rearrange("p (t w) -> p t w", w=5)
                tf = wpool.tile([P, chunk * 5], F32, tag="tf")
                tf3 = tf[:].rearrange("p (t w) -> p t w", w=5)
                tg = wpool.tile([P, chunk * 5], F32, tag="tg")
                tg3 = tg[:].rearrange("p (t w) -> p t w", w=5)
                # even channels 0,2,4,6,8
                nc.vector.tensor_tensor(te3, E0a, bc(al, g0, 5), OP.mult)
                nc.vector.tensor_tensor(tf3, E1a, bc(be, g0, 5), OP.mult)
                nc.vector.tensor_tensor(tg3, E0b, bc(ga, g0, 5), OP.mult)
                nc.vector.tensor_tensor(te3, te3, tf3, OP.add)
                nc.vector.tensor_tensor(
                    _osl(out_t, chunk, 0, 2, 5), te3, tg3, OP.add)
                # odd channels 1,3,5,7 (counts 4)
                E0b4 = _sl(win, chunk, 48, 6, 4)
                E1a4 = _sl(win, chunk, 43, 6, 4)
                E1b4 = _sl(win, chunk, 49, 6, 4)
                te4 = te[:].rearrange("p (t w) -> p t w", w=5)[:, :, 0:4]
                tf4 = tf[:].rearrange("p (t w) -> p t w", w=5)[:, :, 0:4]
                tg4 = tg[:].rearrange("p (t w) -> p t w", w=5)[:, :, 0:4]
                nc.vector.tensor_tensor(te4, E1a4, bc(al, g0, 4), OP.mult)
                nc.vector.tensor_tensor(tf4, E0b4, bc(be, g0, 4), OP.mult)
                nc.vector.tensor_tensor(tg4, E1b4, bc(ga, g0, 4), OP.mult)
                nc.vector.tensor_tensor(te4, te4, tf4, OP.add)
                nc.vector.tensor_tensor(
                    _osl(out_t, chunk, 1, 2, 4), te4, tg4, OP.add)

                nc.sync.dma_start(
                    out=out[:, g0 * CH:(g0 + chunk) * CH], in_=out_t[:])

    nc.compile()
    return nc


def _build_q6(c0, c1, c2, c3):
    r = c0.shape[0]
    w = np.arange(SQ) - PAD
    comps = []
    for arr, idx in ((c0, 2 * w), (c0, 2 * w + 1), (c1, w),
                     (c2, np.floor_divide(w, 2)),
                     (c3, np.floor_divide(w, 4) - 2),
                     (c3, np.floor_divide(w, 4) + 3)):
        m = (idx >= 0) & (idx < arr.shape[1])
        comp = np.zeros((r, SQ), np.float32)
        comp[:, m] = arr[:, idx[m]]
        comps.append(comp)
    return np.stack(comps, axis=-1).reshape(r, SQ * 6)


def make_in_maps(centroids_coords, corr_list, r=R):
    nt = r // P
    c = np.ascontiguousarray(centroids_coords[:, 0], dtype=np.float32).reshape(-1)
    mrow = np.arange(r, dtype=np.float32).reshape(nt, P).T.copy()
    ncores = c.size // r
    in_maps = []
    for k in range(ncores):
        sl = slice(k * r, (k + 1) * r)
        in_maps.append({
            "coords": c[sl].reshape(nt, P).T.copy(),
            "mrow": mrow,
            "q6": _build_q6(*[np.asarray(x[sl], np.float32)
                              for x in corr_list]).ravel(),
        })
    return in_maps


_NC_CACHE = {}
LAST_RESULTS = None


def kernel(centroids_coords, corr0, corr1, corr2, corr3,
           trace=False, tmpdir=None):
    global LAST_RESULTS
    centroids_coords = np.asarray(centroids_coords, dtype=np.float32)
    corrs = [np.asarray(x, dtype=np.float32) for x in (corr0, corr1, corr2, corr3)]
    if "nc" not in _NC_CACHE:
        _NC_CACHE["nc"] = build_nc()
    nc = _NC_CACHE["nc"]
    in_maps = make_in_maps(centroids_coords, corrs)
    res = run_bass_kernel_spmd(nc, in_maps, list(range(NCORES)),
                               trace=trace, tmpdir=tmpdir)
    LAST_RESULTS = res
    parts = []
    for k in range(NCORES):
        o = res.results[k]["out"]
        parts.append(o.reshape(P, NT, CH).transpose(1, 0, 2).reshape(R, CH))
    full = np.concatenate(parts, axis=0)
    return np.ascontiguousarray(
        full.reshape(B, H, W, CH).transpose(0, 3, 1, 2))
